# revision 3
# baseline (speedup 1.0000x reference)
"""Trainium2 Bass kernel for nn_CNNModel (ragged resize -> conv1d -> pools -> MLP -> softmax-mean).

Self-contained: hardcodes shapes B=64, N=256, L=1024, TARGET=100, 8 cores.
Pure data parallel over batch: 8 batches/core, 2048 seqs/core, 16 tiles of
128 seqs. Host precomputes per-seq resize tables (lo, frac) from `lengths`
and folds embed_w into the conv weights via a piecewise-linear feature basis
phi(t) = [1, t, relu(t-2), relu(t-3)]; the device streams all tokens and does:

  GPSIMD indirect_copy gather of (lo,lo+1) pairs -> masked-reduce extract
  -> feature build -> bilinear blend -> PE transpose -> block-Toeplitz conv
  matmul -> maxpool(8) -> dense1+globalmax -> dense2 -> 2-class softmax
  (sigmoid of logit diff) -> per-batch mean via ones-matmul.
"""

import os
import tempfile
from contextlib import ExitStack

import numpy as np

LAST_EXEC_NS = None
LAST_TRACE_DIR = None

B, N, L = 64, 256, 1024
TARGET = 100
KW = 8
NPOOL = 11               # floor(93/8)
N_CORES = 8
SEQ_PER_CORE = (B // N_CORES) * N          # 2048
TILES = SEQ_PER_CORE // 128                # 16
CHUNKS = [(0, 24), (24, 24), (48, 24), (72, 16)]   # (start, n_outputs), pool-aligned

_CACHE = {}


# ----------------------------------------------------------------------------
# host-side helpers
# ----------------------------------------------------------------------------

def _resize_tables(lengths_flat):
    """lo (int, in [0, len-2]) and frac (fp32) per (seq, i) such that
    x_i = e[lo]*(1-frac) + e[lo+1]*frac matches the reference bilinear resize."""
    lf = lengths_flat.astype(np.float64)[:, None]            # [S,1]
    i = np.arange(TARGET, dtype=np.float64)[None, :]
    src = (i + 0.5) * lf / TARGET - 0.5
    f = np.floor(src)
    lo = np.clip(f, 0, lf - 2).astype(np.int64)
    fr = np.clip(src - lo, 0.0, 1.0)
    return lo, fr.astype(np.float32)


def _fold_weights(embed_w, conv_w, conv_b):
    Phi = np.array([[1, 1, 0, 0], [1, 2, 0, 0], [1, 3, 1, 0], [1, 4, 2, 1]],
                   dtype=np.float64)
    E = embed_w[1:5].astype(np.float64)                       # rows t=1..4
    M = np.linalg.solve(Phi, E)                               # [4(f),4(c)]
    wf = np.einsum("fc,kco->kfo", M[1:], conv_w.astype(np.float64))  # [8,3,12]
    bias = conv_b.astype(np.float64) + np.einsum(
        "c,kco->o", M[0], conv_w.astype(np.float64))
    return wf.astype(np.float32), bias.astype(np.float32)


def _conv_chunk_weights(wf):
    """Block-Toeplitz per chunk: rows f-major (f, delta), cols (o_local, ch)."""
    Ws = []
    for (start, opc) in CHUNKS:
        span = opc + KW - 1
        W = np.zeros((3 * span, opc * 12), np.float32)
        for f in range(3):
            for d in range(span):
                for o in range(opc):
                    k = d - o
                    if 0 <= k < KW:
                        W[f * span + d, o * 12:(o + 1) * 12] = wf[k, f, :]
        Ws.append(W)
    return Ws


def _d1_weights(w1):
    RA = np.zeros((72, 192), np.float32)
    RB = np.zeros((60, 160), np.float32)
    for p in range(6):
        RA[12 * p:12 * p + 12, 32 * p:32 * p + 32] = w1
    for p in range(5):
        RB[12 * p:12 * p + 12, 32 * p:32 * p + 32] = w1
    return RA, RB


def _build_host_data(tokens, lengths, embed_w, conv_w, conv_b, w1, b1, w2, b2,
                     wc, bc):
    wf, bconv = _fold_weights(embed_w, conv_w, conv_b)
    Ws = _conv_chunk_weights(wf)
    RA, RB = _d1_weights(w1)
    wcd = (wc[:, 1] - wc[:, 0]).astype(np.float32)
    bcd = float(bc[1] - bc[0])

    consts = {
        "w_c0": Ws[0], "w_c1": Ws[1], "w_c2": Ws[2], "w_c3": Ws[3],
        "r1a": RA, "r1b": RB,
        "w2r": w2.astype(np.float32),
        "maskc": np.stack([(np.arange(16) == (p % 16)) for p in range(128)]
                          ).astype(np.float32),                  # [128,16]
        "ident": np.eye(128, dtype=np.float32),
        "bpool": np.tile(np.tile(bconv, NPOOL)[None, :], (128, 1)),  # [128,132]
        "b1rep": np.tile(b1.astype(np.float32)[None, :], (128, 1)),
        "b2rep": np.tile(b2.astype(np.float32)[None, :], (128, 1)),
        "wcdrep": np.tile(wcd[None, :], (128, 1)),
        "ones1": np.ones((128, 1), np.float32),
        "biasv": np.tile(np.array([-2.0, -3.0, bcd, -bcd, 0.0, 0.0, 0.0, 0.0],
                                  np.float32)[None, :], (128, 1)),
    }

    tokens_r = tokens.reshape(N_CORES, SEQ_PER_CORE, L)
    lengths_r = lengths.reshape(N_CORES, SEQ_PER_CORE)
    per_core = []
    for c in range(N_CORES):
        lo, fr = _resize_tables(lengths_r[c])
        m = dict(consts)
        m["tok"] = np.ascontiguousarray(tokens_r[c])
        m["idx"] = lo.astype(np.uint16).reshape(TILES, 128, TARGET)
        m["frac"] = fr.reshape(TILES, 128, TARGET)
        per_core.append(m)
    return per_core, bcd


# ----------------------------------------------------------------------------
# device program
# ----------------------------------------------------------------------------

def _build_program(bcd, repeat=1, ablate=()):
    import concourse.bass as bass
    import concourse.tile as tile
    from concourse import bacc, mybir

    dt = mybir.dt
    Alu = mybir.AluOpType
    Act = mybir.ActivationFunctionType
    Ax = mybir.AxisListType

    nc = bacc.Bacc("TRN2", target_bir_lowering=False, debug=False)

    def din(name, shape, dtype):
        return nc.dram_tensor(name, shape, dtype, kind="ExternalInput").ap()

    tok_d = din("tok", [SEQ_PER_CORE, L], dt.int32)
    idx_d = din("idx", [TILES, 128, TARGET], dt.uint16)
    frac_d = din("frac", [TILES, 128, TARGET], dt.float32)
    wc_d = [din(f"w_c{i}", [3 * (opc + 7), opc * 12], dt.float32)
            for i, (_, opc) in enumerate(CHUNKS)]
    r1a_d = din("r1a", [72, 192], dt.float32)
    r1b_d = din("r1b", [60, 160], dt.float32)
    w2r_d = din("w2r", [32, 64], dt.float32)
    maskc_d = din("maskc", [128, 16], dt.float32)
    ident_d = din("ident", [128, 128], dt.float32)
    bpool_d = din("bpool", [128, 132], dt.float32)
    b1rep_d = din("b1rep", [128, 32], dt.float32)
    b2rep_d = din("b2rep", [128, 64], dt.float32)
    wcdrep_d = din("wcdrep", [128, 64], dt.float32)
    ones1_d = din("ones1", [128, 1], dt.float32)
    biasv_d = din("biasv", [128, 8], dt.float32)
    out_d = nc.dram_tensor("out", [B // N_CORES, 2], dt.float32,
                           kind="ExternalOutput").ap()

    with tile.TileContext(nc) as tc, ExitStack() as ctx:
        cpool = ctx.enter_context(tc.tile_pool(name="consts", bufs=1))
        iopool = ctx.enter_context(tc.tile_pool(name="io", bufs=2))
        gpool = ctx.enter_context(tc.tile_pool(name="gather", bufs=2))
        wpool = ctx.enter_context(tc.tile_pool(name="work", bufs=2))
        pspool = ctx.enter_context(tc.tile_pool(name="ps", bufs=2, space="PSUM"))
        accpool = ctx.enter_context(tc.tile_pool(name="acc", bufs=1, space="PSUM"))

        def cload(ap_d, shape, dtype, tag):
            t = cpool.tile(shape, dtype, tag=tag)
            nc.sync.dma_start(t[:], ap_d[:])
            return t

        Wc = [cload(wc_d[i], [3 * (opc + 7), opc * 12], dt.float32, f"cW{i}")
              for i, (_, opc) in enumerate(CHUNKS)]
        R1A = cload(r1a_d, [72, 192], dt.float32, "cR1A")
        R1B = cload(r1b_d, [60, 160], dt.float32, "cR1B")
        W2R = cload(w2r_d, [32, 64], dt.float32, "cW2R")
        MASKC = cload(maskc_d, [128, 16], dt.float32, "cMASK")
        IDENT = cload(ident_d, [128, 128], dt.float32, "cID")
        BPOOL = cload(bpool_d, [128, 132], dt.float32, "cBP")
        B1REP = cload(b1rep_d, [128, 32], dt.float32, "cB1")
        B2REP = cload(b2rep_d, [128, 64], dt.float32, "cB2")
        WCDREP = cload(wcdrep_d, [128, 64], dt.float32, "cWCD")
        ONES1 = cload(ones1_d, [128, 1], dt.float32, "cON")
        BIASV = cload(biasv_d, [128, 8], dt.float32, "cBV")

        mean_ps = accpool.tile([2, TILES], dt.float32)

        rep_ctx = tc.For_i(0, repeat, 1) if repeat > 1 else None
        if rep_ctx is not None:
            rep_ctx.__enter__()
        for t in range(TILES):
            # ---- load tile inputs (tokens cast int32->fp32 via SWDGE) ----
            data3 = iopool.tile([128, L // 2, 2], dt.float32, tag="data3")
            nc.gpsimd.dma_start(
                data3[:],
                tok_d[t * 128:(t + 1) * 128, :].rearrange("p (n d) -> p n d", d=2))
            idxt = iopool.tile([128, TARGET], dt.uint16, tag="idxt")
            nc.sync.dma_start(idxt[:], idx_d[t])
            frct = iopool.tile([128, TARGET], dt.float32, tag="frct")
            nc.sync.dma_start(frct[:], frac_d[t])

            # ---- gather pairs: junk[p, i*16+k, e] = tok[p, lo[seq16k, i]+e] ----
            junk = gpool.tile([128, TARGET * 16, 2], dt.float32, tag="junk")
            if "gather" in ablate:
                nc.vector.memset(junk[:].rearrange("p a b -> p (a b)"), 1.0)
            else:
                for (i0, ni) in ((0, 32), (32, 32), (64, 32), (96, 4)):
                    nc.gpsimd.indirect_copy(
                        junk[:, i0 * 16:(i0 + ni) * 16, :],
                        data3[:],
                        idxt[:, i0:i0 + ni],
                        i_know_ap_gather_is_preferred=True)

            # ---- extract tlo/thi via masked grouped reduce ----
            jv = junk[:].rearrange("p (i k) e -> p i k e", k=16)
            mb = MASKC[:].rearrange("p (a k) -> p a k", a=1) \
                         .to_broadcast([128, TARGET, 16])
            prod = wpool.tile([128, TARGET, 16], dt.float32, tag="prod")
            lo3 = wpool.tile([128, 3, TARGET], dt.float32, tag="lo3")
            hi3 = wpool.tile([128, 3, TARGET], dt.float32, tag="hi3")
            nc.vector.tensor_tensor(out=prod[:], in0=jv[:, :, :, 0], in1=mb,
                                    op=Alu.mult)
            nc.vector.tensor_reduce(out=lo3[:, 0, :], in_=prod[:], axis=Ax.X,
                                    op=Alu.add)
            nc.vector.tensor_tensor(out=prod[:], in0=jv[:, :, :, 1], in1=mb,
                                    op=Alu.mult)
            nc.vector.tensor_reduce(out=hi3[:, 0, :], in_=prod[:], axis=Ax.X,
                                    op=Alu.add)

            # ---- features: f1 = t, f2 = relu(t-2), f3 = relu(t-3) ----
            for buf in (lo3, hi3):
                nc.scalar.activation(out=buf[:, 1, :], in_=buf[:, 0, :],
                                     func=Act.Relu, bias=BIASV[:, 0:1])
                nc.scalar.activation(out=buf[:, 2, :], in_=buf[:, 0, :],
                                     func=Act.Relu, bias=BIASV[:, 1:2])

            # ---- bilinear blend: d3 = frac*(hi3 - lo3); x3c per chunk ----
            d3 = wpool.tile([128, 3, TARGET], dt.float32, tag="d3")
            frb = frct[:].rearrange("p (a i) -> p a i", a=1) \
                         .to_broadcast([128, 3, TARGET])
            nc.vector.tensor_tensor(out=d3[:], in0=hi3[:], in1=lo3[:],
                                    op=Alu.subtract)
            nc.vector.tensor_tensor(out=d3[:], in0=d3[:], in1=frb, op=Alu.mult)

            # ---- conv: chunk-contiguous blend -> transpose -> matmul -> pool ----
            pooled = wpool.tile([128, 132], dt.float32, tag="pooled")
            for ci, (start, opc) in enumerate(CHUNKS):
                span = opc + 7
                x3c = wpool.tile([128, 3, span], dt.float32, tag="x3c")
                nc.vector.tensor_tensor(out=x3c[:],
                                        in0=lo3[:, :, start:start + span],
                                        in1=d3[:, :, start:start + span],
                                        op=Alu.add)
                tp = pspool.tile([3 * 31, 128], dt.float32, tag="tp")
                nc.tensor.transpose(
                    out=tp[:3 * span, :],
                    in_=x3c[:].rearrange("p f s -> p (f s)"),
                    identity=IDENT[:])
                xtc = wpool.tile([3 * 31, 128], dt.float32, tag="xtc")
                nc.scalar.copy(out=xtc[:3 * span, :], in_=tp[:3 * span, :])
                y_ps = pspool.tile([128, opc * 12], dt.float32, tag="mm")
                nc.tensor.matmul(out=y_ps[:], lhsT=xtc[:3 * span, :],
                                 rhs=Wc[ci][:], start=True, stop=True)
                g = opc // 8
                yv = y_ps[:].rearrange("p (g o c) -> p g c o", g=g, o=8)
                nc.vector.tensor_reduce(
                    out=pooled[:, 36 * ci:36 * ci + 12 * g]
                        .rearrange("p (g c) -> p g c", g=g),
                    in_=yv, axis=Ax.X, op=Alu.max)

            # ---- bias + relu ----
            h = wpool.tile([128, 132], dt.float32, tag="h")
            nc.vector.tensor_tensor(out=h[:], in0=pooled[:], in1=BPOOL[:],
                                    op=Alu.add)
            nc.scalar.activation(out=h[:], in_=h[:], func=Act.Relu,
                                 bias=BIASV[:, 4:5])

            # ---- dense1 (block-diag) + global max over 11 pools ----
            htA_ps = pspool.tile([72, 128], dt.float32, tag="tp")
            nc.tensor.transpose(out=htA_ps[:], in_=h[:, 0:72], identity=IDENT[:])
            htA = wpool.tile([72, 128], dt.float32, tag="htA")
            nc.scalar.copy(out=htA[:], in_=htA_ps[:])
            htB_ps = pspool.tile([60, 128], dt.float32, tag="tp")
            nc.tensor.transpose(out=htB_ps[:], in_=h[:, 72:132], identity=IDENT[:])
            htB = wpool.tile([60, 128], dt.float32, tag="htB")
            nc.scalar.copy(out=htB[:], in_=htB_ps[:])

            h1a_ps = pspool.tile([128, 192], dt.float32, tag="mm")
            nc.tensor.matmul(out=h1a_ps[:], lhsT=htA[:], rhs=R1A[:],
                             start=True, stop=True)
            h1b_ps = pspool.tile([128, 160], dt.float32, tag="mm2")
            nc.tensor.matmul(out=h1b_ps[:], lhsT=htB[:], rhs=R1B[:],
                             start=True, stop=True)

            ga = wpool.tile([128, 32], dt.float32, tag="ga")
            gb = wpool.tile([128, 32], dt.float32, tag="gb")
            nc.vector.tensor_reduce(
                out=ga[:], in_=h1a_ps[:].rearrange("p (g o) -> p o g", g=6),
                axis=Ax.X, op=Alu.max)
            nc.vector.tensor_reduce(
                out=gb[:], in_=h1b_ps[:].rearrange("p (g o) -> p o g", g=5),
                axis=Ax.X, op=Alu.max)
            nc.vector.tensor_tensor(out=ga[:], in0=ga[:], in1=gb[:], op=Alu.max)
            nc.vector.tensor_tensor(out=ga[:], in0=ga[:], in1=B1REP[:], op=Alu.add)
            nc.scalar.activation(out=ga[:], in_=ga[:], func=Act.Relu,
                                 bias=BIASV[:, 4:5])

            # ---- dense2 + relu ----
            gt_ps = pspool.tile([32, 128], dt.float32, tag="tp")
            nc.tensor.transpose(out=gt_ps[:], in_=ga[:], identity=IDENT[:])
            gt = wpool.tile([32, 128], dt.float32, tag="gt")
            nc.scalar.copy(out=gt[:], in_=gt_ps[:])
            r2_ps = pspool.tile([128, 64], dt.float32, tag="mm2")
            nc.tensor.matmul(out=r2_ps[:], lhsT=gt[:], rhs=W2R[:],
                             start=True, stop=True)
            r2 = wpool.tile([128, 64], dt.float32, tag="r2")
            nc.vector.tensor_tensor(out=r2[:], in0=r2_ps[:], in1=B2REP[:],
                                    op=Alu.add)
            nc.scalar.activation(out=r2[:], in_=r2[:], func=Act.Relu,
                                 bias=BIASV[:, 4:5])

            # ---- classifier: zd = r2@wcd + bcd; probs = [sig(-zd), sig(zd)] ----
            pz = wpool.tile([128, 64], dt.float32, tag="pz")
            nc.vector.tensor_tensor(out=pz[:], in0=r2[:], in1=WCDREP[:],
                                    op=Alu.mult)
            zd = wpool.tile([128, 1], dt.float32, tag="zd")
            nc.vector.tensor_reduce(out=zd[:], in_=pz[:], axis=Ax.X, op=Alu.add)
            probs = wpool.tile([128, 2], dt.float32, tag="probs")
            nc.scalar.activation(out=probs[:, 1:2], in_=zd[:], func=Act.Sigmoid,
                                 bias=BIASV[:, 2:3])
            nc.scalar.activation(out=probs[:, 0:1], in_=zd[:], func=Act.Sigmoid,
                                 bias=BIASV[:, 3:4], scale=-1.0)

            # ---- per-tile node-sum: mean_ps[:, t] = probs.T @ ones ----
            nc.tensor.matmul(out=mean_ps[:, t:t + 1], lhsT=probs[:],
                             rhs=ONES1[:], start=True, stop=True)

        if rep_ctx is not None:
            rep_ctx.__exit__(None, None, None)

        # ---- finalize: combine tile pairs, /256, write [8, 2] ----
        sums = wpool.tile([2, B // N_CORES], dt.float32, tag="sums")
        nc.vector.tensor_reduce(
            out=sums[:], in_=mean_ps[:].rearrange("p (b t) -> p b t", t=2),
            axis=Ax.X, op=Alu.add)
        outs = wpool.tile([2, B // N_CORES], dt.float32, tag="outs")
        nc.scalar.mul(out=outs[:], in_=sums[:], mul=1.0 / N)
        nc.sync.dma_start(out_d.rearrange("b c -> c b"), outs[:])

    nc.compile()
    return nc


# ----------------------------------------------------------------------------
# entry point
# ----------------------------------------------------------------------------

def kernel(**inputs):
    tokens = np.asarray(inputs["tokens"])
    lengths = np.asarray(inputs["lengths"])
    per_core, bcd = _build_host_data(
        tokens, lengths,
        np.asarray(inputs["embed_w"]), np.asarray(inputs["conv_w"]),
        np.asarray(inputs["conv_b"]), np.asarray(inputs["w1"]),
        np.asarray(inputs["b1"]), np.asarray(inputs["w2"]),
        np.asarray(inputs["b2"]), np.asarray(inputs["wc"]),
        np.asarray(inputs["bc"]))

    key = ("prog", round(bcd, 8))
    if key not in _CACHE:
        _CACHE[key] = _build_program(bcd)
    nc = _CACHE[key]

    from concourse.bass_utils import run_bass_kernel_spmd
    trace = os.environ.get("KERNEL_TRACE", "0") == "1"
    tmpdir = tempfile.mkdtemp(prefix="ktrace_") if trace else None
    res = run_bass_kernel_spmd(nc, per_core, list(range(N_CORES)),
                               trace=trace, tmpdir=tmpdir)
    global LAST_EXEC_NS, LAST_TRACE_DIR
    if res.exec_time_ns is not None:
        LAST_EXEC_NS = res.exec_time_ns
        LAST_TRACE_DIR = tmpdir
    out = np.concatenate([res.results[c]["out"] for c in range(N_CORES)], axis=0)
    return out.astype(np.float32)



# revision 4
# speedup vs baseline: 1.0613x; 1.0613x over previous
"""Trainium2 Bass kernel v4: v2 + bf16 PE pipeline + single-sigmoid classifier.

Sequences globally sorted by length, packed into 16-partition groups with at
most two distinct lengths (group-shared gather indices). Tokens ship as uint16
packed pairs. Conv/dense matmuls and transposes run in bf16 (fp32 PSUM
accumulation); select/features are exact in bf16 (small integers). Classifier
computes only p1 = sigmoid(zd + bcd); host recovers p0 = 1 - p1.
"""

import os
import tempfile
from contextlib import ExitStack

import numpy as np
import ml_dtypes

BF16 = ml_dtypes.bfloat16

B, N, L = 64, 256, 1024
TARGET = 100
KW = 8
N_CORES = 8
TILES = 17
PURE_TILES = 9                  # tiles 0..P-1 are single-length-class
NI = 13                         # idx cols used -> 208 gathered slots (200 used)
NIS = 16                        # idx cols stored (32B-aligned stride)
CHUNKS = [(0, 32), (32, 32), (64, 24)]
NPOOL = 11

LAST_EXEC_NS = None
LAST_TRACE_DIR = None

_CACHE = {}


# ----------------------------------------------------------------------------
# host-side helpers
# ----------------------------------------------------------------------------

def _resize_tables(lengths):
    lf = np.asarray(lengths, np.float64)[:, None]
    i = np.arange(TARGET, dtype=np.float64)[None, :]
    src = (i + 0.5) * lf / TARGET - 0.5
    f = np.floor(src)
    lo = np.clip(f, 0, lf - 2).astype(np.int64)
    fr = np.clip(src - lo, 0.0, 1.0)
    return lo, fr.astype(np.float32)


def _fold_weights(embed_w, conv_w, conv_b):
    Phi = np.array([[1, 1, 0, 0], [1, 2, 0, 0], [1, 3, 1, 0], [1, 4, 2, 1]],
                   dtype=np.float64)
    E = embed_w[1:5].astype(np.float64)
    M = np.linalg.solve(Phi, E)
    wf = np.einsum("fc,kco->kfo", M[1:], conv_w.astype(np.float64))  # [8,3,12]
    bias = conv_b.astype(np.float64) + np.einsum(
        "c,kco->o", M[0], conv_w.astype(np.float64))
    return wf.astype(np.float32), bias.astype(np.float32)


def _conv_chunk_weights(wf):
    Ws = []
    for (start, opc) in CHUNKS:
        span = opc + KW - 1
        W = np.zeros((3 * span, opc * 12), np.float32)
        for f in range(3):
            for d in range(span):
                for o in range(opc):
                    k = d - o
                    if 0 <= k < KW:
                        W[f * span + d, o * 12:(o + 1) * 12] = wf[k, f, :]
        Ws.append(W)
    return Ws


def _d1_weights(w1):
    RA = np.zeros((72, 192), np.float32)
    RB = np.zeros((60, 160), np.float32)
    for p in range(6):
        RA[12 * p:12 * p + 12, 32 * p:32 * p + 32] = w1
    for p in range(5):
        RB[12 * p:12 * p + 12, 32 * p:32 * p + 32] = w1
    return RA, RB


def _pack_groups(lengths_flat):
    """Groups of 16 seqs with <=2 distinct lengths.

    Full 16-blocks of one length become pure groups; leftover pieces
    (1..15 seqs of one length) are paired two-per-group (largest with
    smallest fitting) and padded with zero-weight duplicates.
    Returns (groups, purity): each group is a list of 16 (seq_idx, weight).
    """
    from collections import defaultdict
    by_len = defaultdict(list)
    order = np.argsort(lengths_flat, kind="stable")
    for sidx in order:
        by_len[int(lengths_flat[sidx])].append(int(sidx))

    groups, purity = [], []
    pieces = []
    for ln in sorted(by_len):
        seqs = by_len[ln]
        n_full = len(seqs) // 16
        for k in range(n_full):
            groups.append([(s, 1.0) for s in seqs[16 * k:16 * (k + 1)]])
            purity.append(True)
        rem = seqs[16 * n_full:]
        if rem:
            pieces.append(rem)

    pieces.sort(key=len, reverse=True)
    i, j = 0, len(pieces) - 1
    while i <= j:
        cur = [(s, 1.0) for s in pieces[i]]
        if i < j and len(pieces[i]) + len(pieces[j]) <= 16:
            cur += [(s, 1.0) for s in pieces[j]]
            j -= 1
        i += 1
        while len(cur) < 16:
            cur.append((cur[-1][0], 0.0))
        groups.append(cur)
        purity.append(len({int(lengths_flat[s]) for s, _ in cur}) == 1)
    return groups, purity


def _build_host_data(tokens, lengths, embed_w, conv_w, conv_b, w1, b1, w2, b2,
                     wc, bc):
    wf, bconv = _fold_weights(embed_w, conv_w, conv_b)
    Ws = _conv_chunk_weights(wf)
    RA, RB = _d1_weights(w1)
    wcd = (wc[:, 1] - wc[:, 0]).astype(np.float32)
    bcd = float(bc[1] - bc[0])

    tok_flat = tokens.reshape(B * N, L)
    len_flat = lengths.reshape(B * N)
    groups, purity = _pack_groups(len_flat)
    pure_g = [g for g, p in zip(groups, purity) if p]
    mixed_g = [g for g, p in zip(groups, purity) if not p]
    # exactly PURE_TILES*8 pure groups per core; demote the rest to mixed
    n_pure = PURE_TILES * 8 * N_CORES
    if len(pure_g) >= n_pure:
        mixed_g = pure_g[n_pure:] + mixed_g
        pure_g = pure_g[:n_pure]
    else:
        # pad with dummy groups (single class) to fill pure tiles
        dummy = [(pure_g[-1][0][0], 0.0)] * 16 if pure_g else             [(mixed_g[-1][0][0], 0.0)] * 16
        while len(pure_g) < n_pure:
            pure_g.append(list(dummy))
    n_mixed = (TILES - PURE_TILES) * 8 * N_CORES
    assert len(mixed_g) <= n_mixed, (len(mixed_g), n_mixed)
    dummy = [(mixed_g[-1][0][0], 0.0)] * 16
    while len(mixed_g) < n_mixed:
        mixed_g.append(list(dummy))
    # per-core group list: PURE_TILES*8 pure then mixed
    P8 = PURE_TILES * 8
    M8 = (TILES - PURE_TILES) * 8
    groups = []
    for c in range(N_CORES):
        groups.extend(pure_g[c * P8:(c + 1) * P8])
        groups.extend(mixed_g[c * M8:(c + 1) * M8])

    uniq = np.unique(len_flat)
    lo_all, fr_all = _resize_tables(uniq)
    lo_tab = {int(ln): lo_all[k] for k, ln in enumerate(uniq)}
    fr_tab = {int(ln): fr_all[k] for k, ln in enumerate(uniq)}

    tok8 = tok_flat.astype(np.uint16)
    pair16_all = tok8.copy()
    pair16_all[:, :-1] |= tok8[:, 1:] << 8
    pair16_all[:, -1] |= tok8[:, -1] << 8

    const_arrs = {
        "w_c0": Ws[0].astype(BF16), "w_c1": Ws[1].astype(BF16),
        "w_c2": Ws[2].astype(BF16),
        "r1a": RA.astype(BF16), "r1b": RB.astype(BF16),
        "w2r": w2.astype(BF16),
        "identb": np.eye(128, dtype=BF16),
        "bpool": np.tile(np.tile(bconv, NPOOL)[None, :], (128, 1)).astype(BF16),
        "b1rep": np.tile(b1[None, :], (128, 1)).astype(BF16),
        "b2rep": np.tile(b2.astype(np.float32)[None, :], (128, 1)),
        "wcdrep": np.tile(wcd[None, :], (128, 1)),
        "biasv": np.tile(np.array([bcd, 0.0], np.float32)[None, :], (128, 1)),
        "biasvb": np.tile(np.array([-2.0, -3.0, 0.0, 0.0], np.float32)
                          [None, :], (128, 1)).astype(BF16),
    }

    n_tiles_tot = TILES * N_CORES
    pair_t = np.zeros((n_tiles_tot, 128, L), np.uint16)
    idx_t = np.zeros((n_tiles_tot, 128, NIS), np.uint16)
    frac_t = np.zeros((n_tiles_tot, 128, TARGET), BF16)
    mask_t = np.zeros((n_tiles_tot, 128, 2), np.uint8)
    bh_t = np.zeros((n_tiles_tot, 128, B), np.float32)

    for t in range(n_tiles_tot):
        tile_pure = (t % TILES) < PURE_TILES
        for g in range(8):
            grp = groups[t * 8 + g]
            lens_g = [int(len_flat[s]) for s, _ in grp]
            clsA = lens_g[0]
            clsB = next((l for l in lens_g if l != clsA), clsA)
            loA, loB = lo_tab[clsA], lo_tab[clsB]
            if tile_pure:
                assert clsB == clsA, (t, g)
                union = np.concatenate(
                    [loA, np.full(NIS * 16 - TARGET, loA[-1])])
            else:
                union = np.empty(2 * TARGET, np.int64)
                union[0::2] = loA
                union[1::2] = loB
                union = np.concatenate(
                    [union, np.full(NIS * 16 - 2 * TARGET, union[-1])])
            for k in range(16):
                p = 16 * g + k
                seq, w = grp[k]
                ln = int(len_flat[seq])
                pair_t[t, p] = pair16_all[seq]
                idx_t[t, p] = union[k::16]
                frac_t[t, p] = fr_tab[ln].astype(BF16)
                mask_t[t, p, :] = 0 if ln == clsA else 1
                bh_t[t, p, seq // N] = w / N

    per_core = []
    for c in range(N_CORES):
        sl = slice(c * TILES, (c + 1) * TILES)
        arrs = dict(const_arrs)
        arrs["idxa"] = np.ascontiguousarray(
            idx_t[sl].transpose(1, 0, 2)).reshape(128, -1)
        arrs["fraca"] = np.ascontiguousarray(
            frac_t[sl].transpose(1, 0, 2)).reshape(128, -1)
        arrs["selma"] = np.ascontiguousarray(
            mask_t[sl].transpose(1, 0, 2)).reshape(128, -1)
        arrs["bha"] = np.ascontiguousarray(
            bh_t[sl].transpose(1, 0, 2)).reshape(128, -1)
        blob_parts = []
        for name, _shape, _dt, rows in _blob_layout():
            a = arrs[name]
            bview = np.ascontiguousarray(a).view(np.uint8)
            r, nb = bview.shape
            assert r == rows, (name, r, rows)
            if r < 128:
                bview = np.concatenate(
                    [bview, np.zeros((128 - r, nb), np.uint8)], 0)
            pad = (-nb) % 4
            if pad:
                bview = np.concatenate(
                    [bview, np.zeros((128, pad), np.uint8)], 1)
            blob_parts.append(bview)
        m = {"blob": np.concatenate(blob_parts, 1),
             "pairs": np.ascontiguousarray(pair_t[sl])}
        per_core.append(m)
    return per_core, bcd


# ----------------------------------------------------------------------------
# device program
# ----------------------------------------------------------------------------

def _build_program(bcd):
    import concourse.tile as tile
    from concourse import bacc, mybir

    dt = mybir.dt
    Alu = mybir.AluOpType
    Act = mybir.ActivationFunctionType
    Ax = mybir.AxisListType
    bf = dt.bfloat16

    nc = bacc.Bacc("TRN2", target_bir_lowering=False, debug=False)

    def din(name, shape, dtype):
        return nc.dram_tensor(name, shape, dtype, kind="ExternalInput").ap()

    offs, blob_bytes = _blob_offsets()
    pairs_d = din("pairs", [TILES, 128, L], dt.uint16)
    blob_d = din("blob", [128, blob_bytes], dt.uint8)
    out_d = nc.dram_tensor("out", [B, 1], dt.float32,
                           kind="ExternalOutput").ap()

    with tile.TileContext(nc) as tc, ExitStack() as ctx:
        cpool = ctx.enter_context(tc.tile_pool(name="consts", bufs=1))
        iopool = ctx.enter_context(tc.tile_pool(name="io", bufs=4))
        gpool = ctx.enter_context(tc.tile_pool(name="gather", bufs=4))
        wpool = ctx.enter_context(tc.tile_pool(name="work", bufs=3))
        pst = ctx.enter_context(tc.tile_pool(name="pst", bufs=2, space="PSUM"))
        psy = ctx.enter_context(tc.tile_pool(name="psy", bufs=3, space="PSUM"))
        psd = ctx.enter_context(tc.tile_pool(name="psd", bufs=2, space="PSUM"))
        psacc = ctx.enter_context(tc.tile_pool(name="psacc", bufs=1,
                                               space="PSUM"))

        # prefetch first token tiles before the const blob
        pairs_bufs = {}

        def load_pairs(t):
            p = iopool.tile([128, L], dt.uint16, tag="pairs")
            nc.sync.dma_start(p[:], pairs_d[t])
            pairs_bufs[t] = p

        load_pairs(0)
        load_pairs(1)
        load_pairs(2)

        BLOB = cpool.tile([128, blob_bytes], dt.uint8, tag="blob")
        nc.sync.dma_start(BLOB[:], blob_d[:])

        def cview(name, dtype, rows=128):
            off, nb = offs[name]
            return BLOB[0:rows, off:off + nb].bitcast(dtype)

        Wc = [cview(f"w_c{i}", bf, 3 * (opc + 7))
              for i, (_, opc) in enumerate(CHUNKS)]
        R1A = cview("r1a", bf, 72)
        R1B = cview("r1b", bf, 60)
        W2R = cview("w2r", bf, 32)
        IDENTB = cview("identb", bf)
        BPOOL = cview("bpool", bf)
        B1REP = cview("b1rep", bf)
        B2REP = cview("b2rep", dt.float32)
        WCDREP = cview("wcdrep", dt.float32)
        BIASV = cview("biasv", dt.float32)
        BIASVB = cview("biasvb", bf)
        IDXA = cview("idxa", dt.uint16).rearrange("p (t n) -> p t n", n=NIS)
        FRACA = cview("fraca", bf).rearrange("p (t n) -> p t n", n=TARGET)
        SELMA = cview("selma", dt.uint8).rearrange("p (t n) -> p t n", n=2)
        BHA = cview("bha", dt.float32).rearrange("p (t n) -> p t n", n=B)

        acc = psacc.tile([B, 1], dt.float32)

        for t in range(TILES):
            if t + 3 < TILES:
                load_pairs(t + 3)
            pairs = pairs_bufs.pop(t)
            tile_pure = t < PURE_TILES
            nit = 7 if tile_pure else NI

            # ---- gather: junk[p, s] = pairs[p, union[s]] (group-shared) ----
            junk = gpool.tile([128, nit * 16], dt.uint16, tag="junk")
            nc.gpsimd.indirect_copy(
                junk[:], pairs[:], IDXA[:, t, :],
                i_know_ap_gather_is_preferred=True)

            # ---- unpack pair -> lo/hi (u16 bit ops), cast to bf16 ----
            lh16 = gpool.tile([128, 2, nit * 16], dt.uint16, tag="lh16")
            nc.vector.tensor_scalar(out=lh16[:, 0, :], in0=junk[:],
                                    scalar1=255, scalar2=None,
                                    op0=Alu.bitwise_and)
            nc.vector.tensor_scalar(out=lh16[:, 1, :], in0=junk[:],
                                    scalar1=8, scalar2=None,
                                    op0=Alu.logical_shift_right)
            lhf = wpool.tile([128, 2, nit * 16], bf, tag="lhf")
            nc.scalar.copy(out=lhf[:].rearrange("p a b -> p (a b)"),
                           in_=lh16[:].rearrange("p a b -> p (a b)"))

            # ---- select class A/B into feat[:, 0] (exact in bf16) ----
            feat = wpool.tile([128, 3, 2, TARGET], bf, tag="feat")
            sel = feat[:, 0]
            if tile_pure:
                nc.vector.tensor_copy(out=sel, in_=lhf[:, :, :TARGET])
            else:
                lhv = lhf[:].rearrange("p l (i c) -> p l i c", c=2)
                selm = SELMA[:, t, :].rearrange("p (l c) -> p l c", c=1) \
                    .to_broadcast([128, 2, TARGET])
                nc.vector.tensor_copy(out=sel, in_=lhv[:, :, :TARGET, 0])
                nc.vector.copy_predicated(out=sel, mask=selm,
                                          data=lhv[:, :, :TARGET, 1])

            # ---- features: f1 = t, f2 = relu(t-2), f3 = relu(t-3) ----
            nc.scalar.activation(out=feat[:, 1].rearrange("p a b -> p (a b)"),
                                 in_=sel.rearrange("p a b -> p (a b)"),
                                 func=Act.Relu, bias=BIASVB[:, 0:1])
            nc.scalar.activation(out=feat[:, 2].rearrange("p a b -> p (a b)"),
                                 in_=sel.rearrange("p a b -> p (a b)"),
                                 func=Act.Relu, bias=BIASVB[:, 1:2])

            # ---- blend: dif = frac*(hi-lo) ----
            dif = wpool.tile([128, 3, TARGET], bf, tag="dif")
            nc.vector.tensor_tensor(out=dif[:], in0=feat[:, :, 1, :],
                                    in1=feat[:, :, 0, :], op=Alu.subtract)
            frb = FRACA[:, t, :].rearrange("p (a i) -> p a i", a=1) \
                .to_broadcast([128, 3, TARGET])
            nc.vector.tensor_tensor(out=dif[:], in0=dif[:], in1=frb,
                                    op=Alu.mult)

            # ---- conv per chunk: blend-add -> transpose -> matmul -> pool ----
            pooled = wpool.tile([128, 132], bf, tag="pooled")
            for ci, (start, opc) in enumerate(CHUNKS):
                span = opc + 7
                x3c = wpool.tile([128, 3, span], bf, tag=f"x3c{ci}")
                nc.vector.tensor_tensor(
                    out=x3c[:], in0=feat[:, :, 0, start:start + span],
                    in1=dif[:, :, start:start + span], op=Alu.add)
                tp = pst.tile([3 * 39, 128], bf, tag="tp")
                nc.tensor.transpose(
                    out=tp[:3 * span, :],
                    in_=x3c[:].rearrange("p f s -> p (f s)"),
                    identity=IDENTB)
                xtc = wpool.tile([3 * 39, 128], bf, tag=f"xtc{ci}")
                nc.vector.tensor_copy(out=xtc[:3 * span, :],
                                      in_=tp[:3 * span, :])
                y_ps = psy.tile([128, opc * 12], dt.float32, tag="mm")
                nc.tensor.matmul(out=y_ps[:], lhsT=xtc[:3 * span, :],
                                 rhs=Wc[ci], start=True, stop=True)
                g = opc // 8
                yv = y_ps[:].rearrange("p (g o c) -> p g c o", g=g, o=8)
                nc.vector.tensor_reduce(
                    out=pooled[:, 12 * (start // 8):12 * (start // 8 + g)]
                        .rearrange("p (g c) -> p g c", g=g),
                    in_=yv, axis=Ax.X, op=Alu.max)

            # ---- bias + relu ----
            h = wpool.tile([128, 132], bf, tag="h")
            nc.vector.tensor_tensor(out=h[:], in0=pooled[:], in1=BPOOL,
                                    op=Alu.add)
            nc.scalar.activation(out=h[:], in_=h[:], func=Act.Relu,
                                 bias=BIASVB[:, 2:3])

            # ---- dense1 (block-diag) + global max over 11 pools ----
            htA_ps = pst.tile([72, 128], bf, tag="tp")
            nc.tensor.transpose(out=htA_ps[:], in_=h[:, 0:72],
                                identity=IDENTB)
            htA = wpool.tile([72, 128], bf, tag="htA")
            nc.scalar.copy(out=htA[:], in_=htA_ps[:])
            htB_ps = pst.tile([60, 128], bf, tag="tp")
            nc.tensor.transpose(out=htB_ps[:], in_=h[:, 72:132],
                                identity=IDENTB)
            htB = wpool.tile([60, 128], bf, tag="htB")
            nc.scalar.copy(out=htB[:], in_=htB_ps[:])

            h1a_ps = psd.tile([128, 192], dt.float32, tag="mmd")
            nc.tensor.matmul(out=h1a_ps[:], lhsT=htA[:], rhs=R1A,
                             start=True, stop=True)
            h1b_ps = psd.tile([128, 160], dt.float32, tag="mmd")
            nc.tensor.matmul(out=h1b_ps[:], lhsT=htB[:], rhs=R1B,
                             start=True, stop=True)

            ga = wpool.tile([128, 32], bf, tag="ga")
            gb = wpool.tile([128, 32], bf, tag="gb")
            nc.vector.tensor_reduce(
                out=ga[:], in_=h1a_ps[:].rearrange("p (g o) -> p o g", g=6),
                axis=Ax.X, op=Alu.max)
            nc.vector.tensor_reduce(
                out=gb[:], in_=h1b_ps[:].rearrange("p (g o) -> p o g", g=5),
                axis=Ax.X, op=Alu.max)
            nc.vector.tensor_tensor(out=ga[:], in0=ga[:], in1=gb[:],
                                    op=Alu.max)
            nc.vector.tensor_tensor(out=ga[:], in0=ga[:], in1=B1REP,
                                    op=Alu.add)
            nc.scalar.activation(out=ga[:], in_=ga[:], func=Act.Relu,
                                 bias=BIASVB[:, 2:3])

            # ---- dense2 + relu ----
            gt_ps = pst.tile([32, 128], bf, tag="tp")
            nc.tensor.transpose(out=gt_ps[:], in_=ga[:], identity=IDENTB)
            gt = wpool.tile([32, 128], bf, tag="gt")
            nc.scalar.copy(out=gt[:], in_=gt_ps[:])
            r2_ps = psd.tile([128, 64], dt.float32, tag="mmd")
            nc.tensor.matmul(out=r2_ps[:], lhsT=gt[:], rhs=W2R,
                             start=True, stop=True)
            r2 = wpool.tile([128, 64], dt.float32, tag="r2")
            nc.vector.tensor_tensor(out=r2[:], in0=r2_ps[:], in1=B2REP,
                                    op=Alu.add)
            nc.scalar.activation(out=r2[:], in_=r2[:], func=Act.Relu,
                                 bias=BIASV[:, 1:2])

            # ---- classifier: zd = r2@wcd; p1 = sigmoid(zd + bcd) ----
            pz = wpool.tile([128, 64], dt.float32, tag="pz")
            zd = wpool.tile([128, 1], dt.float32, tag="zd")
            nc.vector.tensor_tensor(out=pz[:], in0=r2[:], in1=WCDREP,
                                    op=Alu.mult)
            nc.vector.tensor_reduce(out=zd[:], in_=pz[:], axis=Ax.X,
                                    op=Alu.add)
            probs = wpool.tile([128, 1], dt.float32, tag="probs")
            nc.scalar.activation(out=probs[:], in_=zd[:], func=Act.Sigmoid,
                                 bias=BIASV[:, 0:1])

            # ---- batch scatter: acc[b] += sum_p BH[p, b] * p1[p] ----
            nc.tensor.matmul(out=acc[:], lhsT=BHA[:, t, :],
                             rhs=probs[:], start=(t == 0),
                             stop=(t == TILES - 1))

        outs = wpool.tile([B, 1], dt.float32, tag="outs")
        nc.scalar.copy(out=outs[:], in_=acc[:])
        nc.sync.dma_start(out_d[:], outs[:])

    nc.compile()
    return nc


# ----------------------------------------------------------------------------
# entry point
# ----------------------------------------------------------------------------

def kernel(**inputs):
    tokens = np.asarray(inputs["tokens"])
    lengths = np.asarray(inputs["lengths"])
    per_core, bcd = _build_host_data(
        tokens, lengths,
        np.asarray(inputs["embed_w"]), np.asarray(inputs["conv_w"]),
        np.asarray(inputs["conv_b"]), np.asarray(inputs["w1"]),
        np.asarray(inputs["b1"]), np.asarray(inputs["w2"]),
        np.asarray(inputs["b2"]), np.asarray(inputs["wc"]),
        np.asarray(inputs["bc"]))

    key = ("prog_v6", round(bcd, 8))
    if key not in _CACHE:
        _CACHE[key] = _build_program(bcd)
    nc = _CACHE[key]

    from concourse.bass_utils import run_bass_kernel_spmd
    trace = os.environ.get("KERNEL_TRACE", "0") == "1"
    tmpdir = tempfile.mkdtemp(prefix="ktrace_") if trace else None
    res = run_bass_kernel_spmd(nc, per_core, list(range(N_CORES)),
                               trace=trace, tmpdir=tmpdir)
    global LAST_EXEC_NS, LAST_TRACE_DIR
    if res.exec_time_ns is not None:
        LAST_EXEC_NS = res.exec_time_ns
        LAST_TRACE_DIR = tmpdir
    p1 = np.zeros((B, 1), np.float32)
    for c in range(N_CORES):
        p1 += res.results[c]["out"]
    out = np.concatenate([1.0 - p1, p1], axis=1)
    return out.astype(np.float32)


# revision 5
# speedup vs baseline: 1.1440x; 1.0780x over previous
"""Trainium2 Bass kernel v4: v2 + bf16 PE pipeline + single-sigmoid classifier.

Sequences globally sorted by length, packed into 16-partition groups with at
most two distinct lengths (group-shared gather indices). Tokens ship as uint16
packed pairs. Conv/dense matmuls and transposes run in bf16 (fp32 PSUM
accumulation); select/features are exact in bf16 (small integers). Classifier
computes only p1 = sigmoid(zd + bcd); host recovers p0 = 1 - p1.
"""

import os
import tempfile
from contextlib import ExitStack

import numpy as np
import ml_dtypes

BF16 = ml_dtypes.bfloat16

B, N, L = 64, 256, 1024
TARGET = 100
KW = 8
N_CORES = 8
TILES = 17
PURE_TILES = 9                  # tiles 0..P-1 are single-length-class
NI = 13                         # idx cols used -> 208 gathered slots (200 used)
NIS = 16                        # idx cols stored (32B-aligned stride)
CHUNKS = [(0, 32), (32, 32), (64, 24)]
NPOOL = 11

LAST_EXEC_NS = None
LAST_TRACE_DIR = None

_CACHE = {}


# ----------------------------------------------------------------------------
# host-side helpers
# ----------------------------------------------------------------------------

def _resize_tables(lengths):
    lf = np.asarray(lengths, np.float64)[:, None]
    i = np.arange(TARGET, dtype=np.float64)[None, :]
    src = (i + 0.5) * lf / TARGET - 0.5
    f = np.floor(src)
    lo = np.clip(f, 0, lf - 2).astype(np.int64)
    fr = np.clip(src - lo, 0.0, 1.0)
    return lo, fr.astype(np.float32)


def _fold_weights(embed_w, conv_w, conv_b):
    Phi = np.array([[1, 1, 0, 0], [1, 2, 0, 0], [1, 3, 1, 0], [1, 4, 2, 1]],
                   dtype=np.float64)
    E = embed_w[1:5].astype(np.float64)
    M = np.linalg.solve(Phi, E)
    wf = np.einsum("fc,kco->kfo", M[1:], conv_w.astype(np.float64))  # [8,3,12]
    bias = conv_b.astype(np.float64) + np.einsum(
        "c,kco->o", M[0], conv_w.astype(np.float64))
    return wf.astype(np.float32), bias.astype(np.float32)


def _conv_chunk_weights(wf):
    Ws = []
    for (start, opc) in CHUNKS:
        span = opc + KW - 1
        W = np.zeros((3 * span, opc * 12), np.float32)
        for f in range(3):
            for d in range(span):
                for o in range(opc):
                    k = d - o
                    if 0 <= k < KW:
                        W[f * span + d, o * 12:(o + 1) * 12] = wf[k, f, :]
        Ws.append(W)
    return Ws


def _d1_weights(w1):
    RA = np.zeros((72, 192), np.float32)
    RB = np.zeros((60, 160), np.float32)
    for p in range(6):
        RA[12 * p:12 * p + 12, 32 * p:32 * p + 32] = w1
    for p in range(5):
        RB[12 * p:12 * p + 12, 32 * p:32 * p + 32] = w1
    return RA, RB


def _pack_groups(lengths_flat):
    """Groups of 16 seqs with <=2 distinct lengths.

    Full 16-blocks of one length become pure groups; leftover pieces
    (1..15 seqs of one length) are paired two-per-group (largest with
    smallest fitting) and padded with zero-weight duplicates.
    Returns (groups, purity): each group is a list of 16 (seq_idx, weight).
    """
    from collections import defaultdict
    by_len = defaultdict(list)
    order = np.argsort(lengths_flat, kind="stable")
    for sidx in order:
        by_len[int(lengths_flat[sidx])].append(int(sidx))

    groups, purity = [], []
    pieces = []
    for ln in sorted(by_len):
        seqs = by_len[ln]
        n_full = len(seqs) // 16
        for k in range(n_full):
            groups.append([(s, 1.0) for s in seqs[16 * k:16 * (k + 1)]])
            purity.append(True)
        rem = seqs[16 * n_full:]
        if rem:
            pieces.append(rem)

    pieces.sort(key=len, reverse=True)
    i, j = 0, len(pieces) - 1
    while i <= j:
        cur = [(s, 1.0) for s in pieces[i]]
        if i < j and len(pieces[i]) + len(pieces[j]) <= 16:
            cur += [(s, 1.0) for s in pieces[j]]
            j -= 1
        i += 1
        while len(cur) < 16:
            cur.append((cur[-1][0], 0.0))
        groups.append(cur)
        purity.append(len({int(lengths_flat[s]) for s, _ in cur}) == 1)
    return groups, purity


def _build_host_data(tokens, lengths, embed_w, conv_w, conv_b, w1, b1, w2, b2,
                     wc, bc):
    wf, bconv = _fold_weights(embed_w, conv_w, conv_b)
    Ws = _conv_chunk_weights(wf)
    RA, RB = _d1_weights(w1)
    wcd = (wc[:, 1] - wc[:, 0]).astype(np.float32)
    bcd = float(bc[1] - bc[0])

    tok_flat = tokens.reshape(B * N, L)
    len_flat = lengths.reshape(B * N)
    groups, purity = _pack_groups(len_flat)
    pure_g = [g for g, p in zip(groups, purity) if p]
    mixed_g = [g for g, p in zip(groups, purity) if not p]
    # exactly PURE_TILES*8 pure groups per core; demote the rest to mixed
    n_pure = PURE_TILES * 8 * N_CORES
    if len(pure_g) >= n_pure:
        mixed_g = pure_g[n_pure:] + mixed_g
        pure_g = pure_g[:n_pure]
    else:
        # pad with dummy groups (single class) to fill pure tiles
        dummy = [(pure_g[-1][0][0], 0.0)] * 16 if pure_g else             [(mixed_g[-1][0][0], 0.0)] * 16
        while len(pure_g) < n_pure:
            pure_g.append(list(dummy))
    n_mixed = (TILES - PURE_TILES) * 8 * N_CORES
    assert len(mixed_g) <= n_mixed, (len(mixed_g), n_mixed)
    dummy = [(mixed_g[-1][0][0], 0.0)] * 16
    while len(mixed_g) < n_mixed:
        mixed_g.append(list(dummy))
    # per-core group list: PURE_TILES*8 pure then mixed
    P8 = PURE_TILES * 8
    M8 = (TILES - PURE_TILES) * 8
    groups = []
    for c in range(N_CORES):
        groups.extend(pure_g[c * P8:(c + 1) * P8])
        groups.extend(mixed_g[c * M8:(c + 1) * M8])

    uniq = np.unique(len_flat)
    lo_all, fr_all = _resize_tables(uniq)
    lo_tab = {int(ln): lo_all[k] for k, ln in enumerate(uniq)}
    fr_tab = {int(ln): fr_all[k] for k, ln in enumerate(uniq)}

    tok8 = tok_flat.astype(np.uint16)
    pair16_all = tok8.copy()
    pair16_all[:, :-1] |= tok8[:, 1:] << 8
    pair16_all[:, -1] |= tok8[:, -1] << 8

    const_arrs = {
        "w_c0": Ws[0].astype(BF16), "w_c1": Ws[1].astype(BF16),
        "w_c2": Ws[2].astype(BF16),
        "r1a": RA.astype(BF16), "r1b": RB.astype(BF16),
        "w2r": w2.astype(BF16),
        "identb": np.eye(128, dtype=BF16),
        "bpool": np.tile(np.tile(bconv, NPOOL)[None, :], (128, 1)).astype(BF16),
        "b1rep": np.tile(b1[None, :], (128, 1)).astype(BF16),
        "b2rep": np.tile(b2.astype(np.float32)[None, :], (128, 1)),
        "wcdrep": np.tile(wcd[None, :], (128, 1)),
        "biasv": np.tile(np.array([bcd, 0.0], np.float32)[None, :], (128, 1)),
        "biasvb": np.tile(np.array([-2.0, -3.0, 0.0, 0.0], np.float32)
                          [None, :], (128, 1)).astype(BF16),
    }

    n_tiles_tot = TILES * N_CORES
    pair_t = np.zeros((n_tiles_tot, 128, L), np.uint16)
    idx_t = np.zeros((n_tiles_tot, 128, NIS), np.uint16)
    frac_t = np.zeros((n_tiles_tot, 128, TARGET), BF16)
    mask_t = np.zeros((n_tiles_tot, 128, 2), np.uint8)
    bh_t = np.zeros((n_tiles_tot, 128, B), np.float32)

    for t in range(n_tiles_tot):
        tile_pure = (t % TILES) < PURE_TILES
        for g in range(8):
            grp = groups[t * 8 + g]
            lens_g = [int(len_flat[s]) for s, _ in grp]
            clsA = lens_g[0]
            clsB = next((l for l in lens_g if l != clsA), clsA)
            loA, loB = lo_tab[clsA], lo_tab[clsB]
            if tile_pure:
                assert clsB == clsA, (t, g)
                union = np.concatenate(
                    [loA, np.full(NIS * 16 - TARGET, loA[-1])])
            else:
                union = np.empty(2 * TARGET, np.int64)
                union[0::2] = loA
                union[1::2] = loB
                union = np.concatenate(
                    [union, np.full(NIS * 16 - 2 * TARGET, union[-1])])
            for k in range(16):
                p = 16 * g + k
                seq, w = grp[k]
                ln = int(len_flat[seq])
                pair_t[t, p] = pair16_all[seq]
                idx_t[t, p] = union[k::16]
                frac_t[t, p] = fr_tab[ln].astype(BF16)
                mask_t[t, p, :] = 0 if ln == clsA else 1
                bh_t[t, p, seq // N] = w / N

    per_core = []
    for c in range(N_CORES):
        sl = slice(c * TILES, (c + 1) * TILES)
        arrs = dict(const_arrs)
        arrs["idxa"] = np.ascontiguousarray(
            idx_t[sl].transpose(1, 0, 2)).reshape(128, -1)
        arrs["fraca"] = np.ascontiguousarray(
            frac_t[sl].transpose(1, 0, 2)).reshape(128, -1)
        arrs["selma"] = np.ascontiguousarray(
            mask_t[sl].transpose(1, 0, 2)).reshape(128, -1)
        arrs["bha"] = np.ascontiguousarray(
            bh_t[sl].transpose(1, 0, 2)).reshape(128, -1)
        blob_parts = []
        for name, _shape, _dt, rows in _blob_layout():
            a = arrs[name]
            bview = np.ascontiguousarray(a).view(np.uint8)
            r, nb = bview.shape
            assert r == rows, (name, r, rows)
            if r < 128:
                bview = np.concatenate(
                    [bview, np.zeros((128 - r, nb), np.uint8)], 0)
            pad = (-nb) % 4
            if pad:
                bview = np.concatenate(
                    [bview, np.zeros((128, pad), np.uint8)], 1)
            blob_parts.append(bview)
        m = {"blob": np.concatenate(blob_parts, 1),
             "pairs": np.ascontiguousarray(pair_t[sl])}
        per_core.append(m)
    return per_core, bcd


# ----------------------------------------------------------------------------
# device program
# ----------------------------------------------------------------------------

def _build_program(bcd):
    import concourse.tile as tile
    from concourse import bacc, mybir

    dt = mybir.dt
    Alu = mybir.AluOpType
    Act = mybir.ActivationFunctionType
    Ax = mybir.AxisListType
    bf = dt.bfloat16

    nc = bacc.Bacc("TRN2", target_bir_lowering=False, debug=False)

    def din(name, shape, dtype):
        return nc.dram_tensor(name, shape, dtype, kind="ExternalInput").ap()

    offs, blob_bytes = _blob_offsets()
    pairs_d = din("pairs", [TILES, 128, L], dt.uint16)
    blob_d = din("blob", [128, blob_bytes], dt.uint8)
    out_d = nc.dram_tensor("out", [B, 1], dt.float32,
                           kind="ExternalOutput").ap()

    with tile.TileContext(nc) as tc, ExitStack() as ctx:
        cpool = ctx.enter_context(tc.tile_pool(name="consts", bufs=1))
        iopool = ctx.enter_context(tc.tile_pool(name="io", bufs=4))
        gpool = ctx.enter_context(tc.tile_pool(name="gather", bufs=4))
        wpool = ctx.enter_context(tc.tile_pool(name="work", bufs=4))
        pst = ctx.enter_context(tc.tile_pool(name="pst", bufs=3, space="PSUM"))
        psy = ctx.enter_context(tc.tile_pool(name="psy", bufs=2, space="PSUM"))
        psd = ctx.enter_context(tc.tile_pool(name="psd", bufs=2, space="PSUM"))
        psacc = ctx.enter_context(tc.tile_pool(name="psacc", bufs=1,
                                               space="PSUM"))

        # prefetch first token tiles before the const blob
        pairs_bufs = {}

        def load_pairs(t):
            p = iopool.tile([128, L], dt.uint16, tag="pairs")
            nc.sync.dma_start(p[:], pairs_d[t])
            pairs_bufs[t] = p

        load_pairs(0)
        load_pairs(1)
        load_pairs(2)

        BLOB = cpool.tile([128, blob_bytes], dt.uint8, tag="blob")
        nc.sync.dma_start(BLOB[:], blob_d[:])

        def cview(name, dtype, rows=128):
            off, nb = offs[name]
            return BLOB[0:rows, off:off + nb].bitcast(dtype)

        Wc = [cview(f"w_c{i}", bf, 3 * (opc + 7))
              for i, (_, opc) in enumerate(CHUNKS)]
        R1A = cview("r1a", bf, 72)
        R1B = cview("r1b", bf, 60)
        W2R = cview("w2r", bf, 32)
        IDENTB = cview("identb", bf)
        BPOOL = cview("bpool", bf)
        B1REP = cview("b1rep", bf)
        B2REP = cview("b2rep", dt.float32)
        WCDREP = cview("wcdrep", dt.float32)
        BIASV = cview("biasv", dt.float32)
        BIASVB = cview("biasvb", bf)
        IDXA = cview("idxa", dt.uint16).rearrange("p (t n) -> p t n", n=NIS)
        FRACA = cview("fraca", bf).rearrange("p (t n) -> p t n", n=TARGET)
        SELMA = cview("selma", dt.uint8).rearrange("p (t n) -> p t n", n=2)
        BHA = cview("bha", dt.float32).rearrange("p (t n) -> p t n", n=B)

        acc = psacc.tile([B, 1], dt.float32)

        for t in range(TILES):
            if t + 3 < TILES:
                load_pairs(t + 3)
            pairs = pairs_bufs.pop(t)
            tile_pure = t < PURE_TILES
            nit = 7 if tile_pure else NI

            # ---- gather: junk[p, s] = pairs[p, union[s]] (group-shared) ----
            junk = gpool.tile([128, nit * 16], dt.uint16, tag="junk")
            nc.gpsimd.indirect_copy(
                junk[:], pairs[:], IDXA[:, t, :],
                i_know_ap_gather_is_preferred=True)

            # ---- unpack pair -> lo/hi (u16 bit ops), cast to bf16 ----
            lh16 = gpool.tile([128, 2, nit * 16], dt.uint16, tag="lh16")
            nc.vector.tensor_scalar(out=lh16[:, 0, :], in0=junk[:],
                                    scalar1=255, scalar2=None,
                                    op0=Alu.bitwise_and)
            nc.vector.tensor_scalar(out=lh16[:, 1, :], in0=junk[:],
                                    scalar1=8, scalar2=None,
                                    op0=Alu.logical_shift_right)
            lhf = wpool.tile([128, 2, nit * 16], bf, tag="lhf")
            nc.scalar.copy(out=lhf[:].rearrange("p a b -> p (a b)"),
                           in_=lh16[:].rearrange("p a b -> p (a b)"))

            # ---- select class A/B into feat[:, 0] (exact in bf16) ----
            feat = wpool.tile([128, 3, 2, TARGET], bf, tag="feat")
            sel = feat[:, 0]
            if tile_pure:
                nc.vector.tensor_copy(out=sel, in_=lhf[:, :, :TARGET])
            else:
                lhv = lhf[:].rearrange("p l (i c) -> p l i c", c=2)
                selm = SELMA[:, t, :].rearrange("p (l c) -> p l c", c=1) \
                    .to_broadcast([128, 2, TARGET])
                nc.vector.tensor_copy(out=sel, in_=lhv[:, :, :TARGET, 0])
                nc.vector.copy_predicated(out=sel, mask=selm,
                                          data=lhv[:, :, :TARGET, 1])

            # ---- features: f1 = t, f2 = relu(t-2), f3 = relu(t-3) ----
            nc.scalar.activation(out=feat[:, 1].rearrange("p a b -> p (a b)"),
                                 in_=sel.rearrange("p a b -> p (a b)"),
                                 func=Act.Relu, bias=BIASVB[:, 0:1])
            nc.scalar.activation(out=feat[:, 2].rearrange("p a b -> p (a b)"),
                                 in_=sel.rearrange("p a b -> p (a b)"),
                                 func=Act.Relu, bias=BIASVB[:, 1:2])

            # ---- blend: dif = frac*(hi-lo) ----
            dif = wpool.tile([128, 3, TARGET], bf, tag="dif")
            nc.vector.tensor_tensor(out=dif[:], in0=feat[:, :, 1, :],
                                    in1=feat[:, :, 0, :], op=Alu.subtract)
            frb = FRACA[:, t, :].rearrange("p (a i) -> p a i", a=1) \
                .to_broadcast([128, 3, TARGET])
            nc.vector.tensor_tensor(out=dif[:], in0=dif[:], in1=frb,
                                    op=Alu.mult)

            # ---- conv per chunk: blend-add -> transpose -> matmul -> pool ----
            pooled = wpool.tile([128, 132], bf, tag="pooled")
            for ci, (start, opc) in enumerate(CHUNKS):
                span = opc + 7
                x3c = wpool.tile([128, 3, span], bf, tag=f"x3c{ci}")
                nc.vector.tensor_tensor(
                    out=x3c[:], in0=feat[:, :, 0, start:start + span],
                    in1=dif[:, :, start:start + span], op=Alu.add)
                tp = pst.tile([3 * 39, 128], bf, tag="tp")
                nc.tensor.transpose(
                    out=tp[:3 * span, :],
                    in_=x3c[:].rearrange("p f s -> p (f s)"),
                    identity=IDENTB)
                xtc = wpool.tile([3 * 39, 128], bf, tag=f"xtc{ci}")
                nc.scalar.copy(out=xtc[:3 * span, :], in_=tp[:3 * span, :])
                y_ps = psy.tile([128, opc * 12], dt.float32, tag="mm")
                nc.tensor.matmul(out=y_ps[:], lhsT=xtc[:3 * span, :],
                                 rhs=Wc[ci], start=True, stop=True)
                g = opc // 8
                yv = y_ps[:].rearrange("p (g o c) -> p g c o", g=g, o=8)
                nc.vector.tensor_reduce(
                    out=pooled[:, 12 * (start // 8):12 * (start // 8 + g)]
                        .rearrange("p (g c) -> p g c", g=g),
                    in_=yv, axis=Ax.X, op=Alu.max)

            # ---- bias + relu ----
            h = wpool.tile([128, 132], bf, tag="h")
            nc.vector.tensor_tensor(out=h[:], in0=pooled[:], in1=BPOOL,
                                    op=Alu.add)
            nc.scalar.activation(out=h[:], in_=h[:], func=Act.Relu,
                                 bias=BIASVB[:, 2:3])

            # ---- dense1 (block-diag) + global max over 11 pools ----
            htA_ps = pst.tile([72, 128], bf, tag="tp")
            nc.tensor.transpose(out=htA_ps[:], in_=h[:, 0:72],
                                identity=IDENTB)
            htA = wpool.tile([72, 128], bf, tag="htA")
            nc.scalar.copy(out=htA[:], in_=htA_ps[:])
            htB_ps = pst.tile([60, 128], bf, tag="tp")
            nc.tensor.transpose(out=htB_ps[:], in_=h[:, 72:132],
                                identity=IDENTB)
            htB = wpool.tile([60, 128], bf, tag="htB")
            nc.scalar.copy(out=htB[:], in_=htB_ps[:])

            h1a_ps = psd.tile([128, 192], dt.float32, tag="mmd")
            nc.tensor.matmul(out=h1a_ps[:], lhsT=htA[:], rhs=R1A,
                             start=True, stop=True)
            h1b_ps = psd.tile([128, 160], dt.float32, tag="mmd")
            nc.tensor.matmul(out=h1b_ps[:], lhsT=htB[:], rhs=R1B,
                             start=True, stop=True)

            ga = wpool.tile([128, 32], bf, tag="ga")
            gb = wpool.tile([128, 32], bf, tag="gb")
            nc.vector.tensor_reduce(
                out=ga[:], in_=h1a_ps[:].rearrange("p (g o) -> p o g", g=6),
                axis=Ax.X, op=Alu.max)
            nc.vector.tensor_reduce(
                out=gb[:], in_=h1b_ps[:].rearrange("p (g o) -> p o g", g=5),
                axis=Ax.X, op=Alu.max)
            nc.vector.tensor_tensor(out=ga[:], in0=ga[:], in1=gb[:],
                                    op=Alu.max)
            nc.vector.tensor_tensor(out=ga[:], in0=ga[:], in1=B1REP,
                                    op=Alu.add)
            nc.scalar.activation(out=ga[:], in_=ga[:], func=Act.Relu,
                                 bias=BIASVB[:, 2:3])

            # ---- dense2 + relu ----
            gt_ps = pst.tile([32, 128], bf, tag="tp")
            nc.tensor.transpose(out=gt_ps[:], in_=ga[:], identity=IDENTB)
            gt = wpool.tile([32, 128], bf, tag="gt")
            nc.scalar.copy(out=gt[:], in_=gt_ps[:])
            r2_ps = psd.tile([128, 64], dt.float32, tag="mmd")
            nc.tensor.matmul(out=r2_ps[:], lhsT=gt[:], rhs=W2R,
                             start=True, stop=True)
            r2 = wpool.tile([128, 64], dt.float32, tag="r2")
            nc.vector.tensor_tensor(out=r2[:], in0=r2_ps[:], in1=B2REP,
                                    op=Alu.add)
            nc.scalar.activation(out=r2[:], in_=r2[:], func=Act.Relu,
                                 bias=BIASV[:, 1:2])

            # ---- classifier: zd = r2@wcd; p1 = sigmoid(zd + bcd) ----
            pz = wpool.tile([128, 64], dt.float32, tag="pz")
            zd = wpool.tile([128, 1], dt.float32, tag="zd")
            nc.vector.tensor_tensor(out=pz[:], in0=r2[:], in1=WCDREP,
                                    op=Alu.mult)
            nc.vector.tensor_reduce(out=zd[:], in_=pz[:], axis=Ax.X,
                                    op=Alu.add)
            probs = wpool.tile([128, 1], dt.float32, tag="probs")
            nc.scalar.activation(out=probs[:], in_=zd[:], func=Act.Sigmoid,
                                 bias=BIASV[:, 0:1])

            # ---- batch scatter: acc[b] += sum_p BH[p, b] * p1[p] ----
            nc.tensor.matmul(out=acc[:], lhsT=BHA[:, t, :],
                             rhs=probs[:], start=(t == 0),
                             stop=(t == TILES - 1))

        outs = wpool.tile([B, 1], dt.float32, tag="outs")
        nc.scalar.copy(out=outs[:], in_=acc[:])
        nc.sync.dma_start(out_d[:], outs[:])

    nc.compile()
    return nc


# ----------------------------------------------------------------------------
# entry point
# ----------------------------------------------------------------------------

def kernel(**inputs):
    tokens = np.asarray(inputs["tokens"])
    lengths = np.asarray(inputs["lengths"])
    per_core, bcd = _build_host_data(
        tokens, lengths,
        np.asarray(inputs["embed_w"]), np.asarray(inputs["conv_w"]),
        np.asarray(inputs["conv_b"]), np.asarray(inputs["w1"]),
        np.asarray(inputs["b1"]), np.asarray(inputs["w2"]),
        np.asarray(inputs["b2"]), np.asarray(inputs["wc"]),
        np.asarray(inputs["bc"]))

    key = ("prog_v6", round(bcd, 8))
    if key not in _CACHE:
        _CACHE[key] = _build_program(bcd)
    nc = _CACHE[key]

    from concourse.bass_utils import run_bass_kernel_spmd
    trace = os.environ.get("KERNEL_TRACE", "0") == "1"
    tmpdir = tempfile.mkdtemp(prefix="ktrace_") if trace else None
    res = run_bass_kernel_spmd(nc, per_core, list(range(N_CORES)),
                               trace=trace, tmpdir=tmpdir)
    global LAST_EXEC_NS, LAST_TRACE_DIR
    if res.exec_time_ns is not None:
        LAST_EXEC_NS = res.exec_time_ns
        LAST_TRACE_DIR = tmpdir
    p1 = np.zeros((B, 1), np.float32)
    for c in range(N_CORES):
        p1 += res.results[c]["out"]
    out = np.concatenate([1.0 - p1, p1], axis=1)
    return out.astype(np.float32)
